# revision 37
# baseline (speedup 1.0000x reference)
"""Trainium2 Bass kernel for nn_MoEClassifier (moe_routing) — batch-major rework.

Model (per sample):
  x[16,5] -> flat 80 -> fc1(80->64) gelu -> fc2(64->64) gelu -> LN -> h
  u = user_table[user_id]  (16)
  gate: g_e = sum_r (h @ gU[e])_r * (u @ gV[e])_r + gb_e ; top-2 softmax -> w
  experts (dense): z_e = gelu(h @ e_w1[e] + e_b1[e]); LN(z); lpe = z @ e_w2[e] + e_b2
  logits = sum_e w_e * lpe_e   (10 classes)

Pure data-parallel across 8 NeuronCores (batch 131072 -> 16384/core, 32 tiles
of TN=512).  Key idea: the PE charges a matmul only for its MOVING operand
columns, so wherever a [small x 512]-sample matmul extracts a few per-sample
scalars we swap roles — per-sample activations become the stationary lhsT (one
128-sample block at a time) and the small weight matrix moves.  Output lands
batch-major (samples on partitions), where per-sample scalars are
per-partition scalars:

  - bb-LN stats:   4 swap-matmuls of 2 f32 cols  (from f32 [h2|h2^2] lhsT)
  - gate seg-sum:  4 swap-matmuls of 16 f32 cols (from f32 gprod lhsT)
  - expert fc2:    64 swap-matmuls of 22+2 bf16 cols, with the LN mu-term
    folded into the weights: we2' = g*w2 - (g@w2)/64, so lpe = rs*(z@we2')+cst.
    mu rides as 2 extra weight cols; m2 via 2-col matmuls with z^2 lhsT.
  - cst term:      transpose w to feature-major + 10-col swap-matmul vs cst.
  - combine:       pr = P(psum) * broadcast(ws) on DVE, log-tree reduce on
    Pool, + cst; output DMA'd batch-major (no output transposes).

Precision: gate path is f32 matmuls + fp16 hi/lo splits (fc1/gateA), verified
0 top-2 flips vs f64 on this input set (validate_algebra.py); expert path
bf16.  rel err vs reference: 7.3e-3 (tolerance 2e-2).

PSUM (8 banks): bb 2 (ps1/ps2/psU0/stp rotate), sm 2 (psA/psBA/Abm/psWc/cstD),
z 2 ([128,512] x8 rotate), P 2 (fc2 batch-major halves, freed by pr in P9).
Engine balance (modeled): DVE 257us, Act 254us, Pool 206us, PE 167us ->
298.6us makespan (baseline feature-major version: 383us).

Elementwise stages carry KN_* engine knobs (v=DVE, p=Pool, a=Act) and KB_*
buffer knobs; defaults below are the swept optimum.
"""
import sys, os

for _p in ("/opt/trn_rl_repo",):
    if _p not in sys.path:
        sys.path.insert(0, _p)

import numpy as np
from contextlib import ExitStack

import concourse.bass as bass
import concourse.tile as tile
from concourse import bacc, mybir

F32 = mybir.dt.float32
BF16 = mybir.dt.bfloat16
FP16 = mybir.dt.float16
I32 = mybir.dt.int32
AF = mybir.ActivationFunctionType
ALU = mybir.AluOpType

B = 131072
NCORES = 8
B_CORE = B // NCORES
IN_F = 80
EMB = 64
UDIM = 16
E = 16
RANK = 8
NCLS = 10
NUSERS = 1000
EPS_LN = 1e-5
TN = 512
NCH = TN // 128      # 4 sample-blocks of 128 per tile


def _bc(ap, n):
    """broadcast the (size-1) innermost dim of an AP to n via stride 0"""
    return ap.to_broadcast(list(ap.shape[:-1]) + [n])


# packed constant layouts: name -> (partitions, col offset, col width)
CF32_OFF = {
    "wbb2": (EMB, 0, EMB), "b1": (EMB, 64, 1), "b2": (EMB, 65, 1),
    "stat2": (128, 66, 2), "wgU0": (EMB, 68, 128), "gsum": (128, 196, E),
}
CF32_COLS = 212
CB16_OFF = {
    "identb": (128, 0, 128), "stlb": (3, 128, 128), "we1b": (65, 256, 1024),
    "we2s": (128, 1280, 176), "zw2": (128, 1456, 2), "cstb": (E, 1458, NCLS),
}
CB16_COLS = 1468


def build_program(b_core=B_CORE, mmdt="hybrid", bufs=None):
    ntiles = b_core // TN
    nc = bacc.Bacc("TRN2", target_bir_lowering=False, debug=False,
                   num_devices=NCORES)

    # ---------------- DRAM I/O ----------------
    d_x = nc.dram_tensor("x", [ntiles, IN_F, 2 * TN], FP16, kind="ExternalInput")
    d_u = nc.dram_tensor("u", [ntiles, 128, TN], F32, kind="ExternalInput")
    d_BD = nc.dram_tensor("BD", [ntiles, 128, NCH, 2 * E], F32, kind="ExternalInput")
    d_out = nc.dram_tensor("out", [ntiles, 128, NCH, NCLS], F32, kind="ExternalOutput")

    d_cf32 = nc.dram_tensor("cf32", [128, CF32_COLS], F32, kind="ExternalInput")
    d_cb16 = nc.dram_tensor("cb16", [128, CB16_COLS], BF16, kind="ExternalInput")
    d_ch16 = nc.dram_tensor("ch16", [128, 2 * EMB], FP16, kind="ExternalInput")

    # engine knobs: which engine runs the movable elementwise stages
    # (v=vector, p=pool, a=act where applicable)
    kn = {"z2pool": 3, "stf": "a", "statb": "a", "sa": "a", "h2sq": "v",
          "gcp": "p", "passb": "p", "isw": "v", "w32": "v", "tree": "p",
          "cstt": "v", "wt": "v", "pcls": "a", "prps": 1, "z2act": 2,
          "rsa": "n", "rsb": "n", "zmerge": 1, "iseq": "v", "z2ord": 0}
    for k in list(kn):
        v = os.environ.get("KN_" + k)
        if v is not None:
            kn[k] = int(v) if v.isdigit() else v

    bu = {"inp": 3, "work": 3, "scal": 4, "zsb": 8, "z2sb": 18, "osb": 3,
          "psbb": 2, "pssm": 2, "psz": 1, "psP": 1}
    for k in list(bu):
        v = os.environ.get("KB_" + k)
        if v:
            bu[k] = int(v)
    if bufs:
        bu.update(bufs)

    def eng(sel):
        return {"v": nc.vector, "p": nc.gpsimd}[sel]

    with tile.TileContext(nc) as tc, ExitStack() as ctx:
        cpool = ctx.enter_context(tc.tile_pool(name="consts", bufs=1))
        p_in = ctx.enter_context(tc.tile_pool(name="inp", bufs=bu["inp"]))
        p_w = ctx.enter_context(tc.tile_pool(name="work", bufs=bu["work"]))
        p_sc = ctx.enter_context(tc.tile_pool(name="scal", bufs=bu["scal"]))
        p_z = ctx.enter_context(tc.tile_pool(name="zsb", bufs=bu["zsb"]))
        p_z2 = ctx.enter_context(tc.tile_pool(name="z2sb", bufs=bu["z2sb"]))
        p_out = ctx.enter_context(tc.tile_pool(name="osb", bufs=bu["osb"]))
        ps_bb = ctx.enter_context(tc.tile_pool(name="psbb", bufs=bu["psbb"], space="PSUM"))
        ps_sm = ctx.enter_context(tc.tile_pool(name="pssm", bufs=bu["pssm"], space="PSUM"))
        ps_z = ctx.enter_context(tc.tile_pool(name="psz", bufs=bu["psz"], space="PSUM"))
        ps_P = ctx.enter_context(tc.tile_pool(name="psP", bufs=bu["psP"], space="PSUM"))

        # ------------- constants: packed DMAs, sliced views -------------
        t32 = cpool.tile([128, CF32_COLS], F32, tag="cf32", name="c_f32")
        nc.sync.dma_start(t32[:], d_cf32.ap())
        t16 = cpool.tile([128, CB16_COLS], BF16, tag="cb16", name="c_b16")
        nc.sync.dma_start(t16[:], d_cb16.ap())
        c = {}
        for name, (p, o, w) in CF32_OFF.items():
            c[name] = t32[0:p, o:o + w]
        for name, (p, o, w) in CB16_OFF.items():
            c[name] = t16[0:p, o:o + w]
        t_h16 = cpool.tile([128, 2 * EMB], FP16, tag="ch16", name="c_h16")
        nc.sync.dma_start(t_h16[:], d_ch16.ap())
        c["wbb1h"] = t_h16[0:IN_F, 0:EMB]
        c["wbb1l"] = t_h16[0:IN_F, EMB:2 * EMB]
        c["we1b"] = c["we1b"].rearrange("p (a b) -> p a b", a=8, b=128)
        c["we2s"] = c["we2s"].rearrange("p (a b) -> p a b", a=8, b=22)
        identb = c["identb"]

        def tile_body(it):
            # ==== P0: input DMAs + backbone fc1 + gelu ====
            x_fm = p_in.tile([IN_F, 2 * TN], FP16, tag="x_fm", bufs=3, name=f"x_{it}")
            nc.sync.dma_start(x_fm[:], d_x.ap()[it])
            u_fm = p_in.tile([128, TN], F32, tag="u_fm", bufs=6, name=f"u_{it}")
            nc.sync.dma_start(u_fm[:], d_u.ap()[it])
            BD_t = p_in.tile([128, NCH, 2 * E], F32, tag="BD", bufs=7, name=f"BD_{it}")
            nc.sync.dma_start(BD_t[:], d_BD.ap()[it])
            BT_t = BD_t[:, :, 0:E]
            DT_t = BD_t[:, :, E:2 * E]

            ps1 = ps_bb.tile([EMB, TN], F32, tag="bb", name=f"ps1_{it}")
            nc.tensor.matmul(ps1[:], c["wbb1h"], x_fm[:, 0:TN], start=True, stop=False)
            nc.tensor.matmul(ps1[:], c["wbb1h"], x_fm[:, TN:2 * TN], start=False, stop=False)
            nc.tensor.matmul(ps1[:], c["wbb1l"], x_fm[:, 0:TN], start=False, stop=True)
            h1 = p_w.tile([EMB, TN], F32, tag="h1", bufs=3, name=f"h1_{it}")
            nc.scalar.activation(h1[:], ps1[:], AF.Gelu, bias=c["b1"])

            yield  # ==== P1: backbone fc2 (f32) + gelu ====
            ps2 = ps_bb.tile([EMB, TN], F32, tag="bb", name=f"ps2_{it}")
            nc.tensor.matmul(ps2[:], c["wbb2"], h1[:], start=True, stop=True)
            h2s = p_w.tile([128, TN], F32, tag="h2s", bufs=5, name=f"h2s_{it}")
            nc.scalar.activation(h2s[0:EMB, :], ps2[:], AF.Gelu, bias=c["b2"])

            yield  # ==== P2: h2^2 + bb-LN stats (swap matmuls) ====
            if kn["h2sq"] == "a":
                nc.scalar.activation(h2s[EMB:128, :], h2s[0:EMB, :], AF.Square)
            else:
                eng(kn["h2sq"]).tensor_tensor(h2s[EMB:128, :], h2s[0:EMB, :],
                                              h2s[0:EMB, :], op=ALU.mult)
            psA = ps_sm.tile([128, NCH, 2], F32, tag="sm", name=f"psA_{it}")
            for b in range(NCH):
                nc.tensor.matmul(psA[:, b, :], h2s[:, 128 * b:128 * (b + 1)],
                                 c["stat2"], start=True, stop=True)
            sA = p_sc.tile([128, NCH, 2], F32, tag="sA", bufs=3, name=f"sA_{it}")
            if kn.get("sa", "v") == "a":
                nc.scalar.copy(sA[:], psA[:])
            else:
                nc.vector.tensor_copy(sA[:], psA[:])

            yield  # ==== P3: pass A (bb LN scalars, batch-major) ====
            tmpA = p_sc.tile([128, NCH], F32, tag="tmpA", bufs=3, name=f"tmpA_{it}")
            nc.vector.tensor_tensor(tmpA[:], sA[:, :, 0], sA[:, :, 0], op=ALU.mult)
            vA = p_sc.tile([128, NCH], F32, tag="vA", bufs=3, name=f"vA_{it}")
            nc.vector.scalar_tensor_tensor(vA[:], sA[:, :, 1], EPS_LN, tmpA[:],
                                           op0=ALU.add, op1=ALU.subtract)
            backA = p_sc.tile([128, NCH, 2], F32, tag="backA", bufs=4, name=f"backA_{it}")
            rsA = backA[:, :, 0]
            if kn.get("rsa", "n") == "s":
                sqA = p_sc.tile([128, NCH], F32, tag="sqA", bufs=3, name=f"sqA_{it}")
                nc.scalar.activation(sqA[:], vA[:], AF.Sqrt)
                nc.vector.reciprocal(rsA, sqA[:])
            else:
                _newton_rsqrt(nc, p_sc, vA[:], rsA, [128, NCH], f"nA_{it}", niter=2)
            nc.vector.tensor_tensor(backA[:, :, 1], rsA, sA[:, :, 0], op=ALU.mult)
            backAb = p_sc.tile([128, NCH, 3], BF16, tag="backAb", bufs=3, name=f"backAb_{it}")
            nc.vector.tensor_copy(backAb[:, :, 0:2], backA[:])
            nc.gpsimd.memset(backAb[:, :, 2], 1.0)

            yield  # ==== P4: gate A matmul (f32) + gprod; rs/p transpose + stf ====
            psU0 = ps_bb.tile([128, TN], F32, tag="bb", name=f"psU0_{it}")
            nc.tensor.matmul(psU0[:], c["wgU0"], h2s[0:EMB, :], start=True, stop=True)
            gprod = p_w.tile([128, TN], F32, tag="gprod", bufs=4, name=f"gprod_{it}")
            nc.vector.tensor_tensor(gprod[:], psU0[:], u_fm[:], op=ALU.mult)

            psBA = ps_sm.tile([3, TN], BF16, tag="sm", name=f"psBA_{it}")
            for b in range(NCH):
                nc.tensor.transpose(psBA[:, 128 * b:128 * (b + 1)],
                                    backAb[:, b, :], identb)
            stf = p_sc.tile([3, TN], BF16, tag="stf", bufs=3, name=f"stf_{it}")
            if kn["stf"] == "a":
                nc.scalar.copy(stf[:], psBA[:])
            else:
                nc.vector.tensor_copy(stf[:], psBA[:])

            yield  # ==== P5: gate seg-sum (swap) + g1t; stp broadcast + t1h/hb ====
            A_bm = ps_sm.tile([128, NCH, E], F32, tag="sm", name=f"Abm_{it}")
            for b in range(NCH):
                nc.tensor.matmul(A_bm[:, b, :], gprod[:, 128 * b:128 * (b + 1)],
                                 c["gsum"], start=True, stop=True)
            g1t = p_sc.tile([128, NCH, E], F32, tag="g1t", bufs=3, name=f"g1t_{it}")
            nc.vector.tensor_tensor(g1t[:], A_bm[:], _bc(backA[:, :, 0:1], E),
                                    op=ALU.mult)

            stp = ps_bb.tile([128, TN], F32, tag="bb", name=f"stp_{it}")
            nc.tensor.matmul(stp[:], c["stlb"], stf[:], start=True, stop=True)
            hb = p_w.tile([EMB + 1, TN], BF16, tag="hb", bufs=4, name=f"hb_{it}")
            t1h = p_w.tile([EMB, TN], BF16, tag="t1h", bufs=3, name=f"t1h_{it}")
            nc.vector.tensor_tensor(t1h[:], h2s[0:EMB, :], stp[0:EMB, :], op=ALU.mult)
            nc.vector.tensor_tensor(hb[0:EMB, :], t1h[:], stp[EMB:128, :],
                                    op=ALU.subtract)
            nc.gpsimd.memset(hb[EMB:EMB + 1, :], 1.0)

            yield  # ==== P6: gate g = g1t - p*B + D ====
            g2t = p_sc.tile([128, NCH, E], F32, tag="g2t", bufs=3, name=f"g2t_{it}")
            eng(kn["gcp"]).tensor_tensor(g2t[:], BT_t, _bc(backA[:, :, 1:2], E),
                                         op=ALU.mult)
            g3t = p_sc.tile([128, NCH, E], F32, tag="g3t", bufs=3, name=f"g3t_{it}")
            eng(kn["gcp"]).tensor_tensor(g3t[:], g1t[:], g2t[:], op=ALU.subtract)
            gcp = p_sc.tile([128, NCH, E], F32, tag="gcp", bufs=5, name=f"gcp_{it}")
            eng(kn["gcp"]).tensor_tensor(gcp[:], g3t[:], DT_t, op=ALU.add)

            yield  # ==== P7: experts fc1 + gelu + z^2 ; top-2 gate + cst term ====
            z_sb = []
            if int(kn.get("zmerge", 0)):
                zm_all = []
                for m in range(4):
                    zq = ps_z.tile([128, 2, TN], F32, tag="z", name=f"zq_{it}_{m}")
                    for q in range(2):
                        nc.tensor.matmul(zq[:, q, :], c["we1b"][:, 2 * m + q, :],
                                         hb[:], start=True, stop=True)
                    zm = p_z.tile([128, 2, TN], BF16, tag="z_sb", bufs=bu["zsb"],
                                  name=f"z_{it}_{m}")
                    nc.scalar.activation(zm[:], zq[:], AF.Gelu)
                    zm_all.append(zm)
                z_sb = [zm_all[p // 2][:, p % 2, :] for p in range(8)]
            else:
                for p in range(8):
                    zq = ps_z.tile([128, TN], F32, tag="z", name=f"zq_{it}_{p}")
                    nc.tensor.matmul(zq[:], c["we1b"][:, p, :], hb[:], start=True, stop=True)
                    z = p_z.tile([128, TN], BF16, tag="z_sb", bufs=bu["zsb"],
                                 name=f"z_{it}_{p}")
                    nc.scalar.activation(z[:], zq[:], AF.Gelu)
                    z_sb.append(z)
            z2_sb = []
            na = int(kn.get("z2act", 0))
            np_ = int(kn["z2pool"])
            for p in range(8):
                z2 = p_z2.tile([128, TN], BF16, tag="z2_sb", bufs=bu["z2sb"],
                               name=f"z2_{it}_{p}")
                zsrc = z_sb[p] if int(kn.get("zmerge", 0)) else z_sb[p][:]
                order = int(kn.get("z2ord", 0))
                pp = 7 - p if order else p
                if pp < na:
                    nc.scalar.activation(z2[:], zsrc, AF.Square)
                elif pp < na + np_:
                    nc.gpsimd.tensor_tensor(z2[:], zsrc, zsrc, op=ALU.mult)
                else:
                    nc.vector.tensor_tensor(z2[:], zsrc, zsrc, op=ALU.mult)
                z2_sb.append(z2)

            # top-2 selection (from gcp, P6) and w weights
            vm8 = p_sc.tile([128, NCH, 8], F32, tag="vm8", bufs=3, name=f"vm8_{it}")
            for ch in range(NCH):
                nc.vector.max(vm8[:, ch, :], gcp[:, ch, :])
            dg = p_sc.tile([128, NCH], F32, tag="dg", bufs=3, name=f"dg_{it}")
            nc.vector.tensor_tensor(dg[:], vm8[:, :, 0], vm8[:, :, 1], op=ALU.subtract)
            th = p_sc.tile([128, NCH], F32, tag="th", bufs=3, name=f"th_{it}")
            nc.scalar.activation(th[:], dg[:], AF.Tanh, scale=0.5)
            w12 = p_sc.tile([128, NCH, 2], F32, tag="w12", bufs=3, name=f"w12_{it}")
            nc.vector.tensor_scalar(w12[:, :, 0], th[:], 0.5, 0.5, op0=ALU.mult, op1=ALU.add)
            nc.vector.tensor_scalar(w12[:, :, 1], th[:], -0.5, 0.5, op0=ALU.mult, op1=ALU.add)
            is1 = p_sc.tile([128, NCH, E], F32, tag="is1", bufs=3, name=f"is1_{it}")
            eng(kn.get("iseq", "v")).tensor_tensor(is1[:], gcp[:], _bc(vm8[:, :, 0:1], E),
                                                   op=ALU.is_equal)
            is2 = p_sc.tile([128, NCH, E], F32, tag="is2", bufs=3, name=f"is2_{it}")
            eng(kn.get("iseq", "v")).tensor_tensor(is2[:], gcp[:], _bc(vm8[:, :, 1:2], E),
                                                   op=ALU.is_equal)
            w1t = p_sc.tile([128, NCH, E], F32, tag="w1t", bufs=3, name=f"w1t_{it}")
            eng(kn["isw"]).tensor_tensor(w1t[:], is1[:], _bc(w12[:, :, 0:1], E),
                                         op=ALU.mult)
            w2t = p_sc.tile([128, NCH, E], F32, tag="w2t", bufs=3, name=f"w2t_{it}")
            eng(kn["isw"]).tensor_tensor(w2t[:], is2[:], _bc(w12[:, :, 1:2], E),
                                         op=ALU.mult)
            wsum = p_sc.tile([128, NCH, E], F32, tag="wsum", bufs=4, name=f"wsum_{it}")
            eng(kn["w32"]).tensor_tensor(wsum[:], w1t[:], w2t[:], op=ALU.add)
            # cst term: sum_e w_e * cst[e,c] via transpose + tiny swap-matmul
            wv16 = p_sc.tile([128, NCH, E], BF16, tag="wv16", bufs=3, name=f"wv16_{it}")
            nc.vector.tensor_copy(wv16[:], wsum[:])
            psWc = ps_sm.tile([E, TN], BF16, tag="sm", name=f"psWc_{it}")
            for b in range(NCH):
                nc.tensor.transpose(psWc[:, 128 * b:128 * (b + 1)], wv16[:, b, :],
                                    identb)
            wT = p_sc.tile([E, TN], BF16, tag="wT", bufs=3, name=f"wT_{it}")
            if kn.get("wt", "v") == "a":
                nc.scalar.copy(wT[:], psWc[:])
            else:
                nc.vector.tensor_copy(wT[:], psWc[:])

            yield  # ==== P8: fc2 swap-matmuls + stats + rs + weighted products ====
            cstD = ps_sm.tile([128, NCH, NCLS], F32, tag="sm", name=f"cstD_{it}")
            for b in range(NCH):
                nc.tensor.matmul(cstD[:, b, :], wT[:, 128 * b:128 * (b + 1)],
                                 c["cstb"], start=True, stop=True)
            cstt = p_sc.tile([128, NCH, NCLS], F32, tag="cstt", bufs=4,
                             name=f"cstt_{it}")
            if kn.get("cstt", "v") == "a":
                nc.scalar.copy(cstt[:], cstD[:])
            else:
                nc.vector.tensor_copy(cstt[:], cstD[:])

            # P layout per half: [128, 2 blocks, 256] ; pair p at cols 24p..24p+24
            # cols: 0:10 cls_e0', 10:20 cls_e1', 20 mu_e0, 21 mu_e1, 22:24 m2
            # P psum is intra-phase scratch.
            statB = p_sc.tile([128, NCH, 8, 4], F32, tag="statB", bufs=3,
                              name=f"statB_{it}")
            Pt = ps_P.tile([128, NCH, 256], F32, tag="P", name=f"P_{it}")
            for b in range(NCH):
                for p in range(8):
                    zsl = z_sb[p][:, 128 * b:128 * (b + 1)]
                    z2sl = z2_sb[p][:, 128 * b:128 * (b + 1)]
                    nc.tensor.matmul(Pt[:, b, 24 * p:24 * p + 22], zsl,
                                     c["we2s"][:, p, :], start=True, stop=True)
                    nc.tensor.matmul(Pt[:, b, 24 * p + 22:24 * p + 24], z2sl,
                                     c["zw2"], start=True, stop=True)
            Pv = Pt[:, :, 0:192].rearrange("p b (e k) -> p b e k", e=8, k=24)
            if kn["statb"] == "a":
                nc.scalar.copy(statB[:], Pv[:, :, :, 20:24])
            else:
                nc.vector.tensor_copy(statB[:], Pv[:, :, :, 20:24])
            Pcv = Pv[:, :, :, 0:20].rearrange("p b e (q c) -> p b e q c", q=2, c=10)

            yield  # ==== P9: pass B (rs) + weighted products ====
            muB = statB[:, :, :, 0:2]
            m2B = statB[:, :, :, 2:4]
            tmpB = p_sc.tile([128, NCH, 8, 2], F32, tag="tmpB", bufs=3, name=f"tmpB_{it}")
            eng(kn["passb"]).tensor_tensor(tmpB[:], muB, muB, op=ALU.mult)
            vB = p_sc.tile([128, NCH, 8, 2], F32, tag="vB", bufs=3, name=f"vB_{it}")
            nc.vector.scalar_tensor_tensor(vB[:], m2B, EPS_LN, tmpB[:],
                                           op0=ALU.add, op1=ALU.subtract)
            rsB = p_sc.tile([128, NCH, 8, 2], F32, tag="rsB", bufs=3, name=f"rsB_{it}")
            if kn.get("rsb", "n") == "s":
                sqB = p_sc.tile([128, NCH, 8, 2], F32, tag="sqB", bufs=3,
                                name=f"sqB_{it}")
                nc.scalar.activation(sqB[:], vB[:], AF.Sqrt)
                nc.vector.reciprocal(rsB[:], sqB[:])
            else:
                _newton_rsqrt(nc, p_sc, vB[:], rsB[:], [128, NCH, 8, 2],
                              f"nB_{it}", niter=1)
            rsBf = rsB[:].rearrange("p c e q -> p c (e q)")
            wsb16 = p_sc.tile([128, NCH, E], BF16, tag="wsb16", bufs=3,
                              name=f"wsb16_{it}")
            nc.vector.tensor_tensor(wsb16[:], wsum[:], rsBf, op=ALU.mult)

            wsv = wsb16[:].rearrange("p b (e q o) -> p b e q o", e=8, q=2, o=1)
            pr = p_w.tile([128, NCH, 8, 2, 10], BF16, tag="pr", bufs=6,
                          name=f"pr_{it}")
            nc.vector.tensor_tensor(pr[:], Pcv, _bc(wsv, 10), op=ALU.mult)

            yield  # ==== P10: tree reduce + cst add ====
            osb = p_out.tile([128, NCH, NCLS], F32, tag="osb", bufs=3, name=f"osb_{it}")
            te = eng(kn["tree"]) if kn["tree"] != "a" else nc.vector
            ta = p_sc.tile([128, NCH, 4, 2, 10], BF16, tag="ta", bufs=3,
                           name=f"ta_{it}")
            te.tensor_tensor(ta[:], pr[:, :, 0:4], pr[:, :, 4:8], op=ALU.add)
            tb = p_sc.tile([128, NCH, 2, 2, 10], BF16, tag="tb", bufs=3,
                           name=f"tb_{it}")
            te.tensor_tensor(tb[:], ta[:, :, 0:2], ta[:, :, 2:4], op=ALU.add)
            td = p_sc.tile([128, NCH, 2, 10], BF16, tag="td", bufs=3,
                           name=f"td_{it}")
            te.tensor_tensor(td[:], tb[:, :, 0], tb[:, :, 1], op=ALU.add)
            tf = p_sc.tile([128, NCH, NCLS], BF16, tag="tf", bufs=3,
                           name=f"tf_{it}")
            te.tensor_tensor(tf[:], td[:, :, 0], td[:, :, 1], op=ALU.add)
            nc.vector.tensor_tensor(osb[:], tf[:], cstt[:], op=ALU.add)

            yield  # ==== P11: output DMA ====
            nc.sync.dma_start(d_out.ap()[it], osb[:])

        NPH = 12
        gens = {}
        for k in range(ntiles + NPH - 1):
            if k < ntiles:
                gens[k] = tile_body(k)
            for idx in sorted(gens):
                if next(gens[idx], StopIteration) is StopIteration:
                    del gens[idx]

    nc.compile()
    return nc


def _newton_rsqrt(nc, pool, v_ap, out_ap, shape, tag, niter=2, eng=None):
    """out = 1/sqrt(v) via quake seed + Newton iterations."""
    eng = eng or nc.vector
    r = pool.tile(shape, F32, tag=tag[:3] + "_r", name=tag + "_r")
    t = pool.tile(shape, F32, tag=tag[:3] + "_t", name=tag + "_t")
    eng.tensor_scalar(r[:].bitcast(I32), v_ap.bitcast(I32), 1, None,
                      op0=ALU.logical_shift_right)
    eng.tensor_scalar(r[:].bitcast(I32), r[:].bitcast(I32), -1, 0x5F3759DF,
                      op0=ALU.mult, op1=ALU.add)
    for i in range(niter):
        dst = out_ap if i == niter - 1 else r[:]
        eng.tensor_tensor(t[:], r[:], r[:], op=ALU.mult)
        eng.scalar_tensor_tensor(t[:], t[:], -0.5, v_ap, op0=ALU.mult, op1=ALU.mult)
        eng.scalar_tensor_tensor(dst, t[:], 1.5, r[:], op0=ALU.add, op1=ALU.mult)


# ---------------------------------------------------------------------------
# host-side weight prep
# ---------------------------------------------------------------------------
def prep_consts(inp):
    f = np.float32
    import ml_dtypes
    bf = ml_dtypes.bfloat16
    e_w1, e_b1 = np.asarray(inp["e_w1"], f), np.asarray(inp["e_b1"], f)
    e_g = np.asarray(inp["e_g"], np.float64)
    e_beta = np.asarray(inp["e_beta"], np.float64)
    e_w2, e_b2 = np.asarray(inp["e_w2"], np.float64), np.asarray(inp["e_b2"], np.float64)
    bb_g = np.asarray(inp["bb_g"], np.float64)
    bb_beta = np.asarray(inp["bb_beta"], np.float64)
    gU = np.asarray(inp["gU"], np.float64)

    vals32 = {}
    vals32["wbb2"] = np.asarray(inp["bb_w2"], f)
    vals32["b1"] = np.asarray(inp["bb_b1"], f).reshape(EMB, 1)
    vals32["b2"] = np.asarray(inp["bb_b2"], f).reshape(EMB, 1)
    st = np.zeros((128, 2), f)
    st[0:64, 0] = 1.0 / 64
    st[64:128, 1] = 1.0 / 64
    vals32["stat2"] = st
    wgU0 = np.zeros((EMB, 128), np.float64)
    for e in range(E):
        wgU0[:, e * RANK:(e + 1) * RANK] = gU[e] * bb_g[:, None]
    vals32["wgU0"] = wgU0.astype(f)
    gs = np.zeros((128, E), f)
    for e in range(E):
        gs[e * RANK:(e + 1) * RANK, e] = 1.0
    vals32["gsum"] = gs

    vals16 = {}
    vals16["identb"] = np.eye(128, dtype=f)
    # stp rows: [rs; p; 1] -> stp[0:64]=g*rs ; stp[64:128]=g*p - beta
    stl = np.zeros((3, 128), np.float64)
    stl[0, 0:64] = bb_g
    stl[1, 64:128] = bb_g
    stl[2, 64:128] = -bb_beta
    vals16["stlb"] = stl
    # we1 with bias row 64 (per pair: e0 cols 0:64, e1 cols 64:128)
    we1 = np.zeros((EMB + 1, 8, 128), f)
    for p in range(8):
        we1[0:EMB, p, 0:64] = e_w1[2 * p]
        we1[0:EMB, p, 64:128] = e_w1[2 * p + 1]
        we1[EMB, p, 0:64] = e_b1[2 * p]
        we1[EMB, p, 64:128] = e_b1[2 * p + 1]
    vals16["we1b"] = we1.reshape(EMB + 1, 1024)
    # fc2 swap weights: we2' = g*w2 - (g@w2)/64 ; mu cols 20,21
    gw2 = np.einsum("ed,edc->ec", e_g, e_w2)
    we2n = e_g[:, :, None] * e_w2 - gw2[:, None, :] / 64.0   # [E, 64, 10]
    we2 = np.zeros((128, 8, 22), np.float64)
    for p in range(8):
        e0, e1 = 2 * p, 2 * p + 1
        we2[0:64, p, 0:10] = we2n[e0]
        we2[64:128, p, 10:20] = we2n[e1]
        we2[0:64, p, 20] = 1.0 / 64
        we2[64:128, p, 21] = 1.0 / 64
    vals16["we2s"] = we2.reshape(128, 176)
    zw = np.zeros((128, 2), f)
    zw[0:64, 0] = 1.0 / 64
    zw[64:128, 1] = 1.0 / 64
    vals16["zw2"] = zw
    # xexp [32, 202]: rows 0:16 ws-expansion (0/1), rows 16:32 w->cst cols
    cst = np.einsum("ed,edc->ec", e_beta, e_w2) + e_b2
    vals16["cstb"] = cst

    w1 = np.asarray(inp["bb_w1"], np.float64)
    w1h = w1.astype(np.float16)
    w1l = (w1 - w1h.astype(np.float64)).astype(np.float16)
    ch16 = np.zeros((128, 2 * EMB), np.float16)
    ch16[0:IN_F, 0:EMB] = w1h
    ch16[0:IN_F, EMB:2 * EMB] = w1l

    cf32 = np.zeros((128, CF32_COLS), f)
    for name, (p, o, w) in CF32_OFF.items():
        cf32[0:p, o:o + w] = vals32[name]
    cb16 = np.zeros((128, CB16_COLS), bf)
    for name, (p, o, w) in CB16_OFF.items():
        cb16[0:p, o:o + w] = np.asarray(vals16[name], np.float64).astype(bf)
    return {"cf32": cf32, "cb16": cb16, "ch16": ch16}


def prep_user_tables(inp):
    """uV gather table [NUSERS,128] plus per-user gate tables B, D [NUSERS,E]."""
    gU = np.asarray(inp["gU"], np.float64)
    gV = np.asarray(inp["gV"], np.float64)
    gb = np.asarray(inp["gb"], np.float64)
    ut = np.asarray(inp["ut"], np.float64)
    bb_g = np.asarray(inp["bb_g"], np.float64)
    bb_beta = np.asarray(inp["bb_beta"], np.float64)
    wgU = np.zeros((EMB, 128), np.float64)
    for e in range(E):
        wgU[:, e * RANK:(e + 1) * RANK] = gU[e]
    uV = np.einsum("ud,edr->uer", ut, gV).reshape(NUSERS, 128)  # [u, e*8+r]
    cg = (bb_g @ wgU).reshape(E, RANK)       # wgU^T g
    cb = (bb_beta @ wgU).reshape(E, RANK)    # wgU^T beta
    uV3 = uV.reshape(NUSERS, E, RANK)
    Btab = np.einsum("er,uer->ue", cg, uV3)
    Dtab = np.einsum("er,uer->ue", cb, uV3) + gb[None, :]
    return uV.astype(np.float32), Btab.astype(np.float32), Dtab.astype(np.float32)


def shard_inputs(x, user_ids, inp, b_core):
    """x [B,80] -> per-core [nt,80,1024] fp16 hi|lo feature-major;
    uV gathered+transposed; B/D tables gathered batch-major."""
    ncores = x.shape[0] // b_core
    nt = b_core // TN
    xr = x.astype(np.float64)
    xh = xr.astype(np.float16)
    xl = (xr - xh.astype(np.float64)).astype(np.float16)
    xhs = xh.reshape(ncores, nt, TN, IN_F).transpose(0, 1, 3, 2)
    xls = xl.reshape(ncores, nt, TN, IN_F).transpose(0, 1, 3, 2)
    xs = np.ascontiguousarray(np.concatenate([xhs, xls], axis=3))  # [.., 80, 1024]
    uV, Btab, Dtab = prep_user_tables(inp)
    u = uV[user_ids]                                   # [B, 128]
    us = np.ascontiguousarray(
        u.reshape(ncores, nt, TN, 128).transpose(0, 1, 3, 2))
    # batch-major: sample s at (row=s%128, ch=s//128); B and D side by side
    BD = np.concatenate([Btab[user_ids], Dtab[user_ids]], axis=-1)  # [B, 2E]
    BDg = BD.reshape(ncores, nt, NCH, 128, 2 * E)
    BDt = np.ascontiguousarray(BDg.transpose(0, 1, 3, 2, 4))  # [.., 128, NCH, 2E]
    return xs, us, BDt


_CACHE = {}


def _get_program(b_core, mmdt="hybrid"):
    key = (b_core, mmdt)
    if key not in _CACHE:
        _CACHE[key] = build_program(b_core, mmdt)
    return _CACHE[key]


def build_in_maps(inputs):
    x = np.asarray(inputs["x"], np.float64).reshape(B, IN_F)
    uids = np.asarray(inputs["user_ids"]).astype(np.int64)
    cns = prep_consts({k: np.asarray(v) for k, v in inputs.items()})
    xs, us, BDt = shard_inputs(x, uids, inputs, B_CORE)
    in_maps = []
    for k in range(NCORES):
        m = dict(cns)
        m["x"] = xs[k]
        m["u"] = us[k]
        m["BD"] = BDt[k]
        in_maps.append(m)
    return in_maps


def kernel(**inputs):
    from concourse.bass_utils import run_bass_kernel_spmd
    nc = _get_program(B_CORE)
    in_maps = build_in_maps(inputs)
    res = run_bass_kernel_spmd(nc, in_maps, core_ids=list(range(NCORES)))
    nt = B_CORE // TN
    # out [nt, 128, NCH, NCLS]: sample = it*TN + ch*128 + row
    outs = []
    for r in res.results:
        o = r["out"].reshape(nt, 128, NCH, NCLS).transpose(0, 2, 1, 3)
        outs.append(o.reshape(B_CORE, NCLS))
    return np.concatenate(outs, axis=0).astype(np.float32)
